# revision 32
# baseline (speedup 1.0000x reference)
"""Trainium2 Bass kernel for nn_AttentionGenerator (gnn_message_passing).

Reference math:
    f = einsum('oc,bctv->botv', Wf, feat) + bf          # 1x1 conv, Cout=64
    s_i = einsum('c,bctv->btv', Wa[:64], f)
    s_j = einsum('c,bctv->btv', Wa[64:], f)
    score[b,t,i,j] = s_i[b,t,i] + s_j[b,t,j] + ba
    atten = (exp(leaky_relu(score)) * A) / row_sum

f only enters through the two dot products, so fold Wf/bf/Wa/ba on the
host into u1 = w1@Wf, u2 = w2@Wf (length-256 vectors) and the scalar
c0 = (w1+w2)@bf + ba.  The device computes, per (b,t,v), the two
channel contractions (TensorEngine), an 18x18 broadcast-add + LeakyReLU
+ exp*A + row-normalize.  Memory bound.

Key layout trick ("grouped rotation matmul"): the 128 contraction
partitions are split into G=8 groups of 16.  Each group g owns a
different 1/8 of the (t,v) columns, and over NP=8 accumulation passes
the c-subchunks rotate through the groups (rotation pre-baked into the
host data layout so device APs stay affine; stationary weights are
block-diagonal).  The matmul result s then lands in PSUM as [16, 576]
per batch instead of [2, 4608], which makes the PSUM->SBUF evacuation
8x cheaper (engine cost is free-dim cycles).

Numerics: feat and the folded weights stream in fp8e4 (TRN E4M3), which
halves the dominant HBM stream AND doubles PE throughput via the
DoubleRow perf mode (2 contraction rows/cycle).  Weights are pre-scaled
by 2^11 to clear the fp8 subnormal range; the inverse scale and the
folded bias c0/2 ride the PSUM->SBUF activation copy for free.
Accumulation is fp32 in PSUM; intermediates are bf16; output is written
bf16 and upcast to f32 on the host.

Sharding: pure data parallel - batch B=32 split across 8 NeuronCores
(4 batches each), tiny params replicated, no cross-core comms.
"""

import json
import numpy as np
from contextlib import ExitStack

B, Cin, T, V = 32, 256, 256, 18
NCORES = 8
BPC = B // NCORES  # batches per core
G = 8       # partition groups (16 partitions each)
NP = 8      # rotation passes (c-subchunks of 32 = 16 partitions x 2 fp8 pair)
M = 288     # moving columns per (pass, t-half) = 16 t16 * 18 v
WSCALE = 2048.0  # 2^11 weight pre-scale to clear fp8e4 subnormals

_cached_nc = None


def _legalize_waits_json(bir_json):
    """Split instructions carrying >1 sync wait into single-wait NoOps plus
    the original instruction.  The walrus build in this container accepts at
    most ONE sync-wait command per instruction struct; concourse's Tile
    scheduler freely attaches several.  Hoisting the extra waits onto NoOps
    immediately before the instruction (same engine stream, same position)
    preserves semantics exactly - engines execute their stream in order."""
    bir = json.loads(bir_json)
    ctr = 0
    for fn in bir.get("functions", []):
        for blk in fn.get("blocks", []):
            insts = blk.get("instructions")
            if not insts:
                continue
            out = []
            for inst in insts:
                si = inst.get("sync_info") or {}
                waits = si.get("on_wait") or []
                if len(waits) > 1:
                    for w in waits[:-1]:
                        out.append(
                            {
                                "engine": inst.get("engine"),
                                "ins": [],
                                "name": f"wsplit-{ctr}",
                                "opcode": "NoOp",
                                "outs": [],
                                "sync_info": {"on_update": [], "on_wait": [w]},
                            }
                        )
                        ctr += 1
                    si = dict(si)
                    si["on_wait"] = [waits[-1]]
                    inst = dict(inst)
                    inst["sync_info"] = si
                out.append(inst)
            blk["instructions"] = out
    return json.dumps(bir).encode()


_wait_patch_done = False


def _install_wait_legalizer():
    global _wait_patch_done
    if _wait_patch_done:
        return
    import concourse.bass_utils as bass_utils
    import concourse.bass2jax as bass2jax

    orig = bass_utils.compile_bir_kernel

    def wrapped(bir_json, tmpdir, neff_name="file.neff"):
        return orig(_legalize_waits_json(bir_json), tmpdir, neff_name)

    bass_utils.compile_bir_kernel = wrapped
    bass2jax.compile_bir_kernel = wrapped
    _wait_patch_done = True


def _build_nc(c0_half):
    import concourse.bass as bass
    import concourse.mybir as mybir
    import concourse.tile as tile
    from concourse.alu_op_type import AluOpType

    f32 = mybir.dt.float32
    bf16 = mybir.dt.bfloat16
    fp8 = mybir.dt.float8e4
    nc = bass.Bass(num_swdge_queues=4)
    # feat packed on host: [b, part=(g,c16), pass, pair, thalf, m=(t16,v)]
    feat = nc.dram_tensor("feat", [BPC, 128, NP, 2, 2, M], fp8, kind="ExternalInput")
    # block-diagonal rotated weights: [part=(g,c16), pass, pair, col=(o,g')]
    wmat = nc.dram_tensor("wmat", [128, NP, 2, 2 * G], fp8, kind="ExternalInput")
    amat = nc.dram_tensor("amat", [1, V * V], bf16, kind="ExternalInput")
    out = nc.dram_tensor("out", [BPC, T, V, V], bf16, kind="ExternalOutput")

    with ExitStack() as ctx:
        tc = ctx.enter_context(tile.TileContext(nc))
        singles = ctx.enter_context(tc.tile_pool(name="singles", bufs=1))
        fpool = ctx.enter_context(tc.tile_pool(name="fpool", bufs=1))
        pspool = ctx.enter_context(tc.tile_pool(name="pspool", bufs=2, space="PSUM"))
        spool = ctx.enter_context(tc.tile_pool(name="spool", bufs=2))
        work = ctx.enter_context(tc.tile_pool(name="work", bufs=2))
        opool = ctx.enter_context(tc.tile_pool(name="opool", bufs=2))

        # params ride the ACT HWDGE ring: the sync ring carries only the
        # feat half-DMAs and the small SBUF->SBUF scatters, deliberately
        # ordered so the scatters drain in-line mid-stream with ~2.5
        # batches of feat buffered ahead of each scatter slot
        w_t = singles.tile([128, NP, 2, 2 * G], fp8)
        nc.scalar.dma_start(out=w_t, in_=wmat[:, :, :, :])
        a_bc = singles.tile([128, V * V], bf16)
        nc.scalar.dma_start(out=a_bc, in_=amat[0, :].partition_broadcast(128))

        def f_half(st, h):
            f_q = fpool.tile(
                [128, NP // 2, 2, 2, M], fp8,
                tag=f"f_{st['b']}_{h}", name=f"f_{st['b']}_{h}",
            )
            nc.sync.dma_start(out=f_q, in_=feat[st["b"], :, h * 4 : h * 4 + 4])
            st[f"f{h}"] = f_q

        def mm_tb(st, tb):
            ps = pspool.tile([2 * G, 512], f32, tag=f"ps{tb}", name=f"ps{tb}")
            st[f"ps{tb}"] = ps
            for p in range(NP):
                f_q = st["f0"] if p < 4 else st["f1"]
                nc.tensor.matmul(
                    out=ps[:, 0:M],
                    lhsT=w_t[:, p],
                    rhs=f_q[:, p % 4, :, tb],
                    start=(p == 0),
                    stop=(p == NP - 1),
                    perf_mode=mybir.MatmulPerfMode.DoubleRow,
                )

        def evac_tb(st, tb):
            """PSUM -> SBUF on ACT, folding the 2^-11 weight-scale undo and
            c0/2 (each of s1,s2 carries half so their sum carries c0); the
            APs reorder free (t16,v) so the scatter can group (g t16)."""
            s12 = spool.tile(
                [2 * G, 16, V], bf16, tag=f"s12_{tb}", name=f"s12_{tb}"
            )
            st[f"s12_{tb}"] = s12
            ps = st[f"ps{tb}"]
            psr = bass.AP(
                tensor=ps.tensor, offset=ps.offset,
                ap=[ps.ap[0], [V, 16], [1, V]],
            )
            nc.scalar.activation(
                out=s12, in_=psr,
                func=mybir.ActivationFunctionType.Copy,
                scale=1.0 / WSCALE, bias=c0_half,
            )

        def scatter_tb(st, tb):
            """[16=(o,g), (t16,v)] -> [128=(g,t16), (o,v)] via two in-ring
            sync-queue SBUF->SBUF DMAs (flat element orders match)."""
            s12t = spool.tile(
                [128, 2, V], bf16, tag=f"s12t_{tb}", name=f"s12t_{tb}"
            )
            st[f"s12t_{tb}"] = s12t
            for o in range(2):
                nc.sync.dma_start(
                    out=s12t[:, o], in_=st[f"s12_{tb}"][o * G : (o + 1) * G]
                )

        def score_tb(st, tb):
            """broadcast-add score then LeakyReLU on DVE."""
            s12t = st[f"s12t_{tb}"]
            sc = work.tile([128, V, V], bf16, tag=f"sc{tb}", name=f"sc{tb}")
            ex = work.tile([128, V * V], bf16, tag=f"ex{tb}", name=f"ex{tb}")
            st[f"ex{tb}"] = ex
            s1b = bass.AP(
                tensor=s12t.tensor, offset=s12t.offset,
                ap=[s12t.ap[0], [1, V], [0, V]],
            )
            s2b = bass.AP(
                tensor=s12t.tensor, offset=s12t.offset + V,
                ap=[s12t.ap[0], [0, V], [1, V]],
            )
            nc.vector.tensor_add(out=sc, in0=s1b, in1=s2b)
            scf = bass.AP(
                tensor=sc.tensor, offset=sc.offset, ap=[sc.ap[0], [1, V * V]]
            )
            nc.vector.scalar_tensor_tensor(
                out=ex, in0=scf, scalar=0.1, in1=scf,
                op0=AluOpType.mult, op1=AluOpType.max,
            )

        def exp_tb(st, tb):
            ex = st[f"ex{tb}"]
            nc.scalar.activation(
                out=ex, in_=ex, func=mybir.ActivationFunctionType.Exp
            )

        def mask_tb(st, tb, on_pool):
            exa = work.tile([128, V * V], bf16, tag=f"exa{tb}", name=f"exa{tb}")
            st[f"exa{tb}"] = exa
            abc = bass.AP(
                tensor=a_bc.tensor, offset=a_bc.offset,
                ap=[a_bc.ap[0], [1, V * V]],
            )
            eng = nc.gpsimd if on_pool else nc.vector
            eng.tensor_mul(out=exa, in0=st[f"ex{tb}"], in1=abc)

        def sum_tb(st, tb):
            exa = st[f"exa{tb}"]
            exr = bass.AP(
                tensor=exa.tensor, offset=exa.offset,
                ap=[exa.ap[0], [V, V], [1, V]],
            )
            ssum = work.tile([128, V], bf16, tag=f"ssum{tb}", name=f"ssum{tb}")
            with nc.allow_low_precision(reason="bf16 rowsum of 18 positives"):
                nc.vector.reduce_sum(out=ssum, in_=exr, axis=mybir.AxisListType.X)
            rec = work.tile([128, V], bf16, tag=f"rec{tb}", name=f"rec{tb}")
            st[f"rec{tb}"] = rec
            with nc.allow_low_precision(reason="bf16 reciprocal is plenty"):
                nc.vector.reciprocal(out=rec, in_=ssum)

        def norm_tb(st, tb, on_pool):
            if "att" not in st:
                st["att"] = opool.tile(
                    [128, 2, V, V], bf16, tag="att", name="att"
                )
            att, exa, rec = st["att"], st[f"exa{tb}"], st[f"rec{tb}"]
            exr = bass.AP(
                tensor=exa.tensor, offset=exa.offset,
                ap=[exa.ap[0], [V, V], [1, V]],
            )
            rb = bass.AP(
                tensor=rec.tensor, offset=rec.offset,
                ap=[rec.ap[0], [1, V], [0, V]],
            )
            eng = nc.gpsimd if on_pool else nc.vector
            eng.tensor_mul(out=att[:, tb], in0=exr, in1=rb)

        def out_b(st):
            nc.scalar.dma_start(
                out=out[st["b"], :, :, :].rearrange("(tb p) i j -> p tb i j", p=128),
                in_=st["att"],
            )

        def front(st, tb):
            evac_tb(st, tb)
            scatter_tb(st, tb)

        def chain_a(st, tb):
            score_tb(st, tb)
            exp_tb(st, tb)

        def chain_b(st, tb, on_pool):
            mask_tb(st, tb, on_pool)
            sum_tb(st, tb)
            norm_tb(st, tb, on_pool)

        # Half-batch software pipeline; sync-ring order:
        #   f00 f01 f10 f11 f20 [sc00 sc01] f21 f30 [sc10 sc11] f31
        #   [sc20 sc21] [sc30 sc31]
        sts = [{"b": b} for b in range(BPC)]
        f_half(sts[0], 0)
        f_half(sts[0], 1)
        f_half(sts[1], 0)
        f_half(sts[1], 1)
        f_half(sts[2], 0)
        mm_tb(sts[0], 0)
        mm_tb(sts[0], 1)
        front(sts[0], 0)
        front(sts[0], 1)
        f_half(sts[2], 1)
        f_half(sts[3], 0)
        chain_a(sts[0], 0)
        chain_a(sts[0], 1)
        mm_tb(sts[1], 0)
        mm_tb(sts[1], 1)
        chain_b(sts[0], 0, on_pool=True)
        chain_b(sts[0], 1, on_pool=True)
        out_b(sts[0])
        front(sts[1], 0)
        front(sts[1], 1)
        f_half(sts[3], 1)
        chain_a(sts[1], 0)
        chain_a(sts[1], 1)
        mm_tb(sts[2], 0)
        mm_tb(sts[2], 1)
        chain_b(sts[1], 0, on_pool=True)
        chain_b(sts[1], 1, on_pool=True)
        out_b(sts[1])
        front(sts[2], 0)
        front(sts[2], 1)
        chain_a(sts[2], 0)
        chain_a(sts[2], 1)
        mm_tb(sts[3], 0)
        mm_tb(sts[3], 1)
        chain_b(sts[2], 0, on_pool=True)
        chain_b(sts[2], 1, on_pool=True)
        out_b(sts[2])
        front(sts[3], 0)
        front(sts[3], 1)
        chain_a(sts[3], 0)
        chain_b(sts[3], 0, on_pool=False)
        nc.scalar.dma_start(
            out=out[3, 0:128, :, :], in_=sts[3]["att"][:, 0]
        )
        chain_a(sts[3], 1)
        chain_b(sts[3], 1, on_pool=False)
        nc.scalar.dma_start(
            out=out[3, 128:256, :, :], in_=sts[3]["att"][:, 1]
        )
    return nc


def _prep_params(Wf, bf, Wa, ba):
    import ml_dtypes

    w1, w2 = Wa[:64].astype(np.float64), Wa[64:].astype(np.float64)
    Wf64, bf64 = Wf.astype(np.float64), bf.astype(np.float64)
    u = np.stack([w1 @ Wf64, w2 @ Wf64])  # [2, 256]
    c0 = float(w1 @ bf64 + w2 @ bf64 + float(ba[0]))
    # block-diagonal rotated weights: wmat[(g,c16), p, j, (o,g')] =
    #   u[o, ((g+p)%8)*32 + j*16 + c16] * WSCALE  if g'==g else 0
    # (columns o-major so psum rows for each o are contiguous)
    wm = np.zeros((G, 16, NP, 2, 2, G), dtype=np.float64)
    for g in range(G):
        for p in range(NP):
            sub = (g + p) % NP
            for j in range(2):
                cs = sub * 32 + j * 16 + np.arange(16)
                wm[g, :, p, j, :, g] = u[:, cs].T * WSCALE
    wmat = wm.reshape(128, NP, 2, 2 * G).astype(ml_dtypes.float8_e4m3)
    return wmat, c0 / 2.0


def _pack_feat(feat_core):
    """[bpc, 256, 256, 18] f32 -> [bpc, 128, NP, 2, 2, M] fp8e4 with the
    rotation pre-baked: part=(g,c16), free=(pass, pair, thalf, t16, v) holds
    feat[c = ((g+pass)%8)*32 + pair*16 + c16, t = thalf*128 + g*16 + t16, v].
    """
    import ml_dtypes

    bpc = feat_core.shape[0]
    a8 = feat_core.astype(ml_dtypes.float8_e4m3)
    # c = sub*32 + j*16 + c16 ; t = tb*128 + g*16 + t16
    a = a8.reshape(bpc, NP, 2, 16, 2, G, 16, V)  # b, sub, j, c16, tb, g, t16, v
    a = a.transpose(0, 5, 3, 1, 2, 4, 6, 7)  # b, g, c16, sub, j, tb, t16, v
    packed = np.empty((bpc, G, 16, NP, 2, 2, 16, V), dtype=ml_dtypes.float8_e4m3)
    for g in range(G):
        packed[:, g] = a[:, g][:, :, (g + np.arange(NP)) % NP]
    return np.ascontiguousarray(packed.reshape(bpc, 128, NP, 2, 2, M))


def get_nc(c0_half):
    global _cached_nc
    if _cached_nc is None:
        _cached_nc = _build_nc(c0_half)
    return _cached_nc


def kernel(feat, A, Wf, bf, Wa, ba):
    _install_wait_legalizer()
    from concourse.bass_utils import run_bass_kernel_spmd

    import ml_dtypes

    feat = np.asarray(feat, dtype=np.float32)
    A = (
        np.ascontiguousarray(np.asarray(A, dtype=np.float32))
        .reshape(1, V * V)
        .astype(ml_dtypes.bfloat16)
    )
    wmat, c0_half = _prep_params(
        np.asarray(Wf, np.float32),
        np.asarray(bf, np.float32),
        np.asarray(Wa, np.float32),
        np.asarray(ba, np.float32),
    )

    nc = get_nc(c0_half)
    in_maps = [
        {
            "feat": _pack_feat(feat[i * BPC : (i + 1) * BPC]),
            "wmat": wmat,
            "amat": A,
        }
        for i in range(NCORES)
    ]
    res = run_bass_kernel_spmd(nc, in_maps, core_ids=list(range(NCORES)))
    return np.concatenate(
        [np.asarray(r["out"]).astype(np.float32) for r in res.results], axis=0
    )


# revision 33
# speedup vs baseline: 1.0506x; 1.0506x over previous
"""Trainium2 Bass kernel for nn_AttentionGenerator (gnn_message_passing).

Reference math:
    f = einsum('oc,bctv->botv', Wf, feat) + bf          # 1x1 conv, Cout=64
    s_i = einsum('c,bctv->btv', Wa[:64], f)
    s_j = einsum('c,bctv->btv', Wa[64:], f)
    score[b,t,i,j] = s_i[b,t,i] + s_j[b,t,j] + ba
    atten = (exp(leaky_relu(score)) * A) / row_sum

f only enters through the two dot products, so fold Wf/bf/Wa/ba on the
host into u1 = w1@Wf, u2 = w2@Wf (length-256 vectors) and the scalar
c0 = (w1+w2)@bf + ba.  The device computes, per (b,t,v), the two
channel contractions (TensorEngine), an 18x18 broadcast-add + LeakyReLU
+ exp*A + row-normalize.  Memory bound.

Key layout trick ("grouped rotation matmul"): the 128 contraction
partitions are split into G=8 groups of 16.  Each group g owns a
different 1/8 of the (t,v) columns, and over NP=8 accumulation passes
the c-subchunks rotate through the groups (rotation pre-baked into the
host data layout so device APs stay affine; stationary weights are
block-diagonal).  The matmul result s then lands in PSUM as [16, 576]
per batch instead of [2, 4608], which makes the PSUM->SBUF evacuation
8x cheaper (engine cost is free-dim cycles).

Numerics: feat and the folded weights stream in fp8e4 (TRN E4M3), which
halves the dominant HBM stream AND doubles PE throughput via the
DoubleRow perf mode (2 contraction rows/cycle).  Weights are pre-scaled
by 2^11 to clear the fp8 subnormal range; the inverse scale and the
folded bias c0/2 ride the PSUM->SBUF activation copy for free.
Accumulation is fp32 in PSUM; intermediates are bf16; output is written
bf16 and upcast to f32 on the host.

Sharding: pure data parallel - batch B=32 split across 8 NeuronCores
(4 batches each), tiny params replicated, no cross-core comms.
"""

import json
import numpy as np
from contextlib import ExitStack

B, Cin, T, V = 32, 256, 256, 18
NCORES = 8
BPC = B // NCORES  # batches per core
G = 8       # partition groups (16 partitions each)
NP = 8      # rotation passes (c-subchunks of 32 = 16 partitions x 2 fp8 pair)
M = 288     # moving columns per (pass, t-half) = 16 t16 * 18 v
WSCALE = 2048.0  # 2^11 weight pre-scale to clear fp8e4 subnormals

_cached_nc = None


def _legalize_waits_json(bir_json):
    """Split instructions carrying >1 sync wait into single-wait NoOps plus
    the original instruction.  The walrus build in this container accepts at
    most ONE sync-wait command per instruction struct; concourse's Tile
    scheduler freely attaches several.  Hoisting the extra waits onto NoOps
    immediately before the instruction (same engine stream, same position)
    preserves semantics exactly - engines execute their stream in order."""
    bir = json.loads(bir_json)
    ctr = 0
    for fn in bir.get("functions", []):
        for blk in fn.get("blocks", []):
            insts = blk.get("instructions")
            if not insts:
                continue
            out = []
            for inst in insts:
                si = inst.get("sync_info") or {}
                waits = si.get("on_wait") or []
                if len(waits) > 1:
                    for w in waits[:-1]:
                        out.append(
                            {
                                "engine": inst.get("engine"),
                                "ins": [],
                                "name": f"wsplit-{ctr}",
                                "opcode": "NoOp",
                                "outs": [],
                                "sync_info": {"on_update": [], "on_wait": [w]},
                            }
                        )
                        ctr += 1
                    si = dict(si)
                    si["on_wait"] = [waits[-1]]
                    inst = dict(inst)
                    inst["sync_info"] = si
                out.append(inst)
            blk["instructions"] = out
    return json.dumps(bir).encode()


_wait_patch_done = False


def _install_wait_legalizer():
    global _wait_patch_done
    if _wait_patch_done:
        return
    import concourse.bass_utils as bass_utils
    import concourse.bass2jax as bass2jax

    orig = bass_utils.compile_bir_kernel

    def wrapped(bir_json, tmpdir, neff_name="file.neff"):
        return orig(_legalize_waits_json(bir_json), tmpdir, neff_name)

    bass_utils.compile_bir_kernel = wrapped
    bass2jax.compile_bir_kernel = wrapped
    _wait_patch_done = True


def _build_nc(c0_half):
    import concourse.bass as bass
    import concourse.mybir as mybir
    import concourse.tile as tile
    from concourse.alu_op_type import AluOpType

    f32 = mybir.dt.float32
    bf16 = mybir.dt.bfloat16
    fp8 = mybir.dt.float8e4
    nc = bass.Bass(num_swdge_queues=4)
    # feat packed on host: [b, part=(g,c16), pass, pair, thalf, m=(t16,v)]
    feat = nc.dram_tensor("feat", [BPC, 128, NP, 2, 2, M], fp8, kind="ExternalInput")
    # block-diagonal rotated weights: [part=(g,c16), pass, pair, col=(o,g')]
    wmat = nc.dram_tensor("wmat", [128, NP, 2, 2 * G], fp8, kind="ExternalInput")
    amat = nc.dram_tensor("amat", [1, V * V], bf16, kind="ExternalInput")
    out = nc.dram_tensor("out", [BPC, T, V, V], bf16, kind="ExternalOutput")

    with ExitStack() as ctx:
        tc = ctx.enter_context(tile.TileContext(nc))
        singles = ctx.enter_context(tc.tile_pool(name="singles", bufs=1))
        fpool = ctx.enter_context(tc.tile_pool(name="fpool", bufs=1))
        pspool = ctx.enter_context(tc.tile_pool(name="pspool", bufs=2, space="PSUM"))
        wpsum = ctx.enter_context(tc.tile_pool(name="wpsum", bufs=1, space="PSUM"))
        spool = ctx.enter_context(tc.tile_pool(name="spool", bufs=2))
        work = ctx.enter_context(tc.tile_pool(name="work", bufs=2))
        opool = ctx.enter_context(tc.tile_pool(name="opool", bufs=2))

        # params ride the ACT HWDGE ring: the sync ring carries only the
        # feat half-DMAs and the small SBUF->SBUF scatters, deliberately
        # ordered so the scatters drain in-line mid-stream with ~2.5
        # batches of feat buffered ahead of each scatter slot
        w_t = singles.tile([128, NP, 2, 2 * G], fp8)
        nc.scalar.dma_start(out=w_t, in_=wmat[:, :, :, :])
        a_bc = singles.tile([128, V * V], bf16)
        nc.scalar.dma_start(out=a_bc, in_=amat[0, :].partition_broadcast(128))

        # PE clock warm-up: throwaway DoubleRow matmuls on zeroed SBUF ramp
        # the PE out of its low-power state before the first real matmul.
        # Dedicated PSUM pool - an unread tile in the shared pool would
        # block its buffer ring and serialize the real matmuls.
        junk = singles.tile([128, 2, M], fp8)
        nc.gpsimd.memset(junk, 0)
        jw = singles.tile([128, 2, 2 * G], fp8)
        nc.gpsimd.memset(jw, 0)
        wps = wpsum.tile([2 * G, 512], f32)
        for _ in range(8):
            nc.tensor.matmul(
                out=wps[:, 0:M],
                lhsT=jw,
                rhs=junk,
                start=True,
                stop=True,
                perf_mode=mybir.MatmulPerfMode.DoubleRow,
            )

        def f_half(st, h):
            f_q = fpool.tile(
                [128, NP // 2, 2, 2, M], fp8,
                tag=f"f_{st['b']}_{h}", name=f"f_{st['b']}_{h}",
            )
            nc.sync.dma_start(out=f_q, in_=feat[st["b"], :, h * 4 : h * 4 + 4])
            st[f"f{h}"] = f_q

        def mm_tb(st, tb):
            ps = pspool.tile([2 * G, 512], f32, tag=f"ps{tb}", name=f"ps{tb}")
            st[f"ps{tb}"] = ps
            for p in range(NP):
                f_q = st["f0"] if p < 4 else st["f1"]
                nc.tensor.matmul(
                    out=ps[:, 0:M],
                    lhsT=w_t[:, p],
                    rhs=f_q[:, p % 4, :, tb],
                    start=(p == 0),
                    stop=(p == NP - 1),
                    perf_mode=mybir.MatmulPerfMode.DoubleRow,
                )

        def evac_tb(st, tb):
            """PSUM -> SBUF on ACT, folding the 2^-11 weight-scale undo and
            c0/2 (each of s1,s2 carries half so their sum carries c0); the
            APs reorder free (t16,v) so the scatter can group (g t16)."""
            s12 = spool.tile(
                [2 * G, 16, V], bf16, tag=f"s12_{tb}", name=f"s12_{tb}"
            )
            st[f"s12_{tb}"] = s12
            ps = st[f"ps{tb}"]
            psr = bass.AP(
                tensor=ps.tensor, offset=ps.offset,
                ap=[ps.ap[0], [V, 16], [1, V]],
            )
            nc.scalar.activation(
                out=s12, in_=psr,
                func=mybir.ActivationFunctionType.Copy,
                scale=1.0 / WSCALE, bias=c0_half,
            )

        def scatter_tb(st, tb):
            """[16=(o,g), (t16,v)] -> [128=(g,t16), (o,v)] via two in-ring
            sync-queue SBUF->SBUF DMAs (flat element orders match)."""
            s12t = spool.tile(
                [128, 2, V], bf16, tag=f"s12t_{tb}", name=f"s12t_{tb}"
            )
            st[f"s12t_{tb}"] = s12t
            for o in range(2):
                nc.sync.dma_start(
                    out=s12t[:, o], in_=st[f"s12_{tb}"][o * G : (o + 1) * G]
                )

        def score_tb(st, tb):
            """broadcast-add score then LeakyReLU on DVE."""
            s12t = st[f"s12t_{tb}"]
            sc = work.tile([128, V, V], bf16, tag=f"sc{tb}", name=f"sc{tb}")
            ex = work.tile([128, V * V], bf16, tag=f"ex{tb}", name=f"ex{tb}")
            st[f"ex{tb}"] = ex
            s1b = bass.AP(
                tensor=s12t.tensor, offset=s12t.offset,
                ap=[s12t.ap[0], [1, V], [0, V]],
            )
            s2b = bass.AP(
                tensor=s12t.tensor, offset=s12t.offset + V,
                ap=[s12t.ap[0], [0, V], [1, V]],
            )
            nc.vector.tensor_add(out=sc, in0=s1b, in1=s2b)
            scf = bass.AP(
                tensor=sc.tensor, offset=sc.offset, ap=[sc.ap[0], [1, V * V]]
            )
            nc.vector.scalar_tensor_tensor(
                out=ex, in0=scf, scalar=0.1, in1=scf,
                op0=AluOpType.mult, op1=AluOpType.max,
            )

        def exp_tb(st, tb):
            ex = st[f"ex{tb}"]
            nc.scalar.activation(
                out=ex, in_=ex, func=mybir.ActivationFunctionType.Exp
            )

        def mask_tb(st, tb, on_pool):
            exa = work.tile([128, V * V], bf16, tag=f"exa{tb}", name=f"exa{tb}")
            st[f"exa{tb}"] = exa
            abc = bass.AP(
                tensor=a_bc.tensor, offset=a_bc.offset,
                ap=[a_bc.ap[0], [1, V * V]],
            )
            eng = nc.gpsimd if on_pool else nc.vector
            eng.tensor_mul(out=exa, in0=st[f"ex{tb}"], in1=abc)

        def sum_tb(st, tb):
            exa = st[f"exa{tb}"]
            exr = bass.AP(
                tensor=exa.tensor, offset=exa.offset,
                ap=[exa.ap[0], [V, V], [1, V]],
            )
            ssum = work.tile([128, V], bf16, tag=f"ssum{tb}", name=f"ssum{tb}")
            with nc.allow_low_precision(reason="bf16 rowsum of 18 positives"):
                nc.vector.reduce_sum(out=ssum, in_=exr, axis=mybir.AxisListType.X)
            rec = work.tile([128, V], bf16, tag=f"rec{tb}", name=f"rec{tb}")
            st[f"rec{tb}"] = rec
            with nc.allow_low_precision(reason="bf16 reciprocal is plenty"):
                nc.vector.reciprocal(out=rec, in_=ssum)

        def norm_tb(st, tb, on_pool):
            if "att" not in st:
                st["att"] = opool.tile(
                    [128, 2, V, V], bf16, tag="att", name="att"
                )
            att, exa, rec = st["att"], st[f"exa{tb}"], st[f"rec{tb}"]
            exr = bass.AP(
                tensor=exa.tensor, offset=exa.offset,
                ap=[exa.ap[0], [V, V], [1, V]],
            )
            rb = bass.AP(
                tensor=rec.tensor, offset=rec.offset,
                ap=[rec.ap[0], [1, V], [0, V]],
            )
            eng = nc.gpsimd if on_pool else nc.vector
            eng.tensor_mul(out=att[:, tb], in0=exr, in1=rb)

        def out_b(st):
            nc.scalar.dma_start(
                out=out[st["b"], :, :, :].rearrange("(tb p) i j -> p tb i j", p=128),
                in_=st["att"],
            )

        def front(st, tb):
            evac_tb(st, tb)
            scatter_tb(st, tb)

        def chain_a(st, tb):
            score_tb(st, tb)
            exp_tb(st, tb)

        def chain_b(st, tb, on_pool):
            mask_tb(st, tb, on_pool)
            sum_tb(st, tb)
            norm_tb(st, tb, on_pool)

        # Half-batch software pipeline; sync-ring order:
        #   f00 f01 f10 f11 f20 [sc00 sc01] f21 f30 [sc10 sc11] f31
        #   [sc20 sc21] [sc30 sc31]
        sts = [{"b": b} for b in range(BPC)]
        f_half(sts[0], 0)
        f_half(sts[0], 1)
        f_half(sts[1], 0)
        f_half(sts[1], 1)
        f_half(sts[2], 0)
        mm_tb(sts[0], 0)
        mm_tb(sts[0], 1)
        front(sts[0], 0)
        front(sts[0], 1)
        f_half(sts[2], 1)
        f_half(sts[3], 0)
        chain_a(sts[0], 0)
        chain_a(sts[0], 1)
        mm_tb(sts[1], 0)
        mm_tb(sts[1], 1)
        chain_b(sts[0], 0, on_pool=True)
        chain_b(sts[0], 1, on_pool=True)
        out_b(sts[0])
        front(sts[1], 0)
        front(sts[1], 1)
        f_half(sts[3], 1)
        chain_a(sts[1], 0)
        chain_a(sts[1], 1)
        mm_tb(sts[2], 0)
        mm_tb(sts[2], 1)
        chain_b(sts[1], 0, on_pool=True)
        chain_b(sts[1], 1, on_pool=True)
        out_b(sts[1])
        front(sts[2], 0)
        front(sts[2], 1)
        chain_a(sts[2], 0)
        chain_a(sts[2], 1)
        mm_tb(sts[3], 0)
        mm_tb(sts[3], 1)
        chain_b(sts[2], 0, on_pool=True)
        chain_b(sts[2], 1, on_pool=True)
        out_b(sts[2])
        front(sts[3], 0)
        front(sts[3], 1)
        chain_a(sts[3], 0)
        chain_b(sts[3], 0, on_pool=False)
        nc.scalar.dma_start(
            out=out[3, 0:128, :, :], in_=sts[3]["att"][:, 0]
        )
        chain_a(sts[3], 1)
        chain_b(sts[3], 1, on_pool=False)
        nc.scalar.dma_start(
            out=out[3, 128:256, :, :], in_=sts[3]["att"][:, 1]
        )
    return nc


def _prep_params(Wf, bf, Wa, ba):
    import ml_dtypes

    w1, w2 = Wa[:64].astype(np.float64), Wa[64:].astype(np.float64)
    Wf64, bf64 = Wf.astype(np.float64), bf.astype(np.float64)
    u = np.stack([w1 @ Wf64, w2 @ Wf64])  # [2, 256]
    c0 = float(w1 @ bf64 + w2 @ bf64 + float(ba[0]))
    # block-diagonal rotated weights: wmat[(g,c16), p, j, (o,g')] =
    #   u[o, ((g+p)%8)*32 + j*16 + c16] * WSCALE  if g'==g else 0
    # (columns o-major so psum rows for each o are contiguous)
    wm = np.zeros((G, 16, NP, 2, 2, G), dtype=np.float64)
    for g in range(G):
        for p in range(NP):
            sub = (g + p) % NP
            for j in range(2):
                cs = sub * 32 + j * 16 + np.arange(16)
                wm[g, :, p, j, :, g] = u[:, cs].T * WSCALE
    wmat = wm.reshape(128, NP, 2, 2 * G).astype(ml_dtypes.float8_e4m3)
    return wmat, c0 / 2.0


def _pack_feat(feat_core):
    """[bpc, 256, 256, 18] f32 -> [bpc, 128, NP, 2, 2, M] fp8e4 with the
    rotation pre-baked: part=(g,c16), free=(pass, pair, thalf, t16, v) holds
    feat[c = ((g+pass)%8)*32 + pair*16 + c16, t = thalf*128 + g*16 + t16, v].
    """
    import ml_dtypes

    bpc = feat_core.shape[0]
    a8 = feat_core.astype(ml_dtypes.float8_e4m3)
    # c = sub*32 + j*16 + c16 ; t = tb*128 + g*16 + t16
    a = a8.reshape(bpc, NP, 2, 16, 2, G, 16, V)  # b, sub, j, c16, tb, g, t16, v
    a = a.transpose(0, 5, 3, 1, 2, 4, 6, 7)  # b, g, c16, sub, j, tb, t16, v
    packed = np.empty((bpc, G, 16, NP, 2, 2, 16, V), dtype=ml_dtypes.float8_e4m3)
    for g in range(G):
        packed[:, g] = a[:, g][:, :, (g + np.arange(NP)) % NP]
    return np.ascontiguousarray(packed.reshape(bpc, 128, NP, 2, 2, M))


def get_nc(c0_half):
    global _cached_nc
    if _cached_nc is None:
        _cached_nc = _build_nc(c0_half)
    return _cached_nc


def kernel(feat, A, Wf, bf, Wa, ba):
    _install_wait_legalizer()
    from concourse.bass_utils import run_bass_kernel_spmd

    import ml_dtypes

    feat = np.asarray(feat, dtype=np.float32)
    A = (
        np.ascontiguousarray(np.asarray(A, dtype=np.float32))
        .reshape(1, V * V)
        .astype(ml_dtypes.bfloat16)
    )
    wmat, c0_half = _prep_params(
        np.asarray(Wf, np.float32),
        np.asarray(bf, np.float32),
        np.asarray(Wa, np.float32),
        np.asarray(ba, np.float32),
    )

    nc = get_nc(c0_half)
    in_maps = [
        {
            "feat": _pack_feat(feat[i * BPC : (i + 1) * BPC]),
            "wmat": wmat,
            "amat": A,
        }
        for i in range(NCORES)
    ]
    res = run_bass_kernel_spmd(nc, in_maps, core_ids=list(range(NCORES)))
    return np.concatenate(
        [np.asarray(r["out"]).astype(np.float32) for r in res.results], axis=0
    )
